# revision 39
# baseline (speedup 1.0000x reference)
"""Trainium2 Bass kernel: conv2d(64->128, 3x3, valid) + bias + mish(mish(.)).

Full inputs:  x [8, 64, 256, 256] f32, weight [128, 64, 3, 3] f32, bias [128] f32
Full output:  y [8, 128, 254, 254] f32

Sharding: data-parallel over batch, image n -> NeuronCore n (8 cores).

Per-core strategy:
  * SBUF x layout is parity-split: partitions 0-63 hold (cin, even rows),
    partitions 64-127 hold (cin, odd rows), both as [cin, i, col] with the
    same free offset for row pair (2i, 2i+1).  A 3x3 conv tap pair
    (kh, kh+1) then contracts over all 128 partitions in ONE matmul, and
    the leftover tap is a 64-deep matmul, so each 2-row output block is
    6 matmuls (3 pair + 3 single) of free size 2x254=508 accumulated in
    one PSUM bank: 4.5 "full" matmuls of work in 6 instructions.
  * Matmuls run in float32r (fp32 with mantissa rounded to 11 bits; inputs
    pre-rounded on host) which streams at bf16 rate for free dims >= 256.
  * mish(mish(y+bias)) is computed as x*t with t = (1-q^2)/(1+q^2),
    q = sigmoid(-x) (an exact identity: t = tanh(softplus(x))).  The two
    sigmoids run on ScalarE (real HW table); the divide runs on VectorE via
    two 7-stage custom DVE ops (bitwise-NOT reciprocal seed + two
    Newton-Raphson steps, ~1e-5 rel); the bias add is fused into a custom
    multiply op reading PSUM directly; the final multiply runs on GpSimd.
  * Output rows processed in 8 chunks of 32 (last chunk overlaps 2 rows so
    every chunk/block/group has identical shape).
"""

import sys

sys.path.insert(0, "/opt/trn_rl_repo")

import numpy as np

import concourse.bass as bass
import concourse.mybir as mybir
import concourse.tile as tile
from concourse import bacc
from concourse.bass_utils import run_bass_kernel_spmd

F32 = mybir.dt.float32
F32R = mybir.dt.float32r
BF16 = mybir.dt.bfloat16
AFT = mybir.ActivationFunctionType

CIN, H, W = 64, 256, 256
COUT, KHW = 128, 3
HO, WO = 254, 254
NCORES = 8
NI = 17  # row-pairs held in SBUF per chunk (34 input rows)

# chunk starts; every chunk computes 32 output rows (last overlaps by 2)
CHUNKS = [0, 32, 64, 96, 128, 160, 192, 222]


def _patch_act_tables():
    """concourse's act-table map drops functions living in the generic 'act2'
    pwp slot, so Mish (present on TRN2 in the mish_and_others set) looks
    unavailable.  Re-add it for this process."""
    import concourse.hw_specs as hw_specs
    import concourse.bacc as bacc_mod

    if getattr(bacc_mod, "_mish_patch", False):
        return
    orig = hw_specs.get_activation_tables

    def patched(module_arch):
        t = dict(orig(module_arch))
        if "mish_and_others" in t:
            t["mish_and_others"] = set(t["mish_and_others"]) | {AFT.Mish}
        return t

    bacc_mod.get_activation_tables = patched
    bacc_mod._mish_patch = True


def fp32r_round(a: np.ndarray) -> np.ndarray:
    """Round fp32 to the fp32r grid (mantissa to 11 explicit bits, RNE).
    Matches walrus's fp32_to_fp32r bit-exactly for finite values."""
    a = np.ascontiguousarray(a, dtype=np.float32)
    b = a.view(np.uint32)
    r = (b + np.uint32(0x7FF) + ((b >> np.uint32(12)) & np.uint32(1))) & np.uint32(
        0xFFFFF000
    )
    return r.view(np.float32)


_CUSTOM_OPS = {}


def _register_custom_ops():
    """Register two custom DVE ops used for mish:
      MISH_RECIP_A(q)      = y1 ~ 1/(1+q^2)   (NOT-seed + 1 NR)
      MISH_RECIP_B(q, y1)  = 1 - 2*y2, y2 = NR(y1)  == -(1-q^2)/(1+q^2) = -tanh(softplus)
    mish(x) = x*t with t = (1-q^2)/(1+q^2), q = sigmoid(-x).
    """
    if _CUSTOM_OPS:
        return _CUSTOM_OPS
    import re as _re

    import concourse.dve_ops as dv
    from concourse.dve_spec import AluOp, Bin, Spec, Src0, Src1, C0, C1, C2

    from concourse.dve_spec import One

    def _refA(in0, in1, c0, c1, c2):
        d = in0 * in0 + np.float32(1.0)
        n = (~d.view(np.int32)).view(np.float32)
        y0 = n * c0
        return y0 * (c1 - d * y0)

    def _refB(in0, in1, c0, c1, c2):
        d = in0 * in0 + np.float32(1.0)
        y2 = in1 * (c0 - d * in1)
        return (np.float32(1.0) - in0 * in0) * y2

    def _refM(in0, in1, c0, c1, c2):
        in0 = in0.reshape(in0.shape[0], -1)
        in1 = in1.reshape(in1.shape[0], -1)
        if isinstance(c0, np.ndarray):
            c0 = c0.reshape(c0.shape[0], 1)
        return (in0 + c0) * in1

    _dA = Src0 * Src0 + One
    _nA = Bin(AluOp.BITWISE_NOT, _dA, _dA)
    _y0 = _nA * C0
    bodyA = _y0 * (C1 - _dA * _y0)

    _dB = Src0 * Src0 + One
    _y2 = Src1 * (C0 - _dB * Src1)
    bodyB = (One - Src0 * Src0) * _y2

    bodyM = (Src0 + C0) * Src1

    def _mk(name, body, ref):
        spec = Spec(body=body, reference=ref)
        op = dv.DveOp(name, spec, subdim=False, uops_sha={})
        # register row so compile() can resolve the opcode
        if name not in dv._SUB_OPCODE_FOR_NAME:
            dv._SUB_OPCODE_FOR_NAME[name] = max(dv._SUB_OPCODE_FOR_NAME.values()) + 1
        try:
            op.compile("v3")
        except ValueError as e:
            m = _re.search(r'uops_sha\["v3"\]="([0-9a-f]+)"', str(e))
            assert m, f"no sha in: {e}"
            op = dv.DveOp(name, spec, subdim=False, uops_sha={"v3": m.group(1)})
        op.compile("v3")
        dv.OPS.append(op)
        dv.CUSTOM_DVE_SPECS[name] = spec
        return op

    # T2: from s = q^2 (bounded [0, 0.345] because mish >= -0.309):
    # d = s+1; y0 = C0 + C1*d (linear minimax seed of 1/d on [1,1.345]);
    # y1 = y0*(C2 - d*y0) (one Newton step, C2=2); out = (1-s)*y1 = t2.
    def _refT(in0, in1, c0, c1, c2):
        d = in0 + np.float32(1.0)
        y0 = c0 + c1 * d
        y1 = y0 * (c2 - d * y0)
        return (np.float32(1.0) - in0) * y1

    _dT = Src0 + One
    _y0T = C0 + C1 * _dT
    _y1T = _y0T * (C2 - _dT * _y0T)
    bodyT = (One - Src0) * _y1T

    # P1/P2: one fused op per mish.  t = (1-s)/(1+s) with s = q^2 is
    # approximated by the minimax cubic Q(s) = 1 + s*(C0 + s*(C1 + s*C2))
    # (constant term pinned at 1), and the product with the pre-activation
    # is folded into the op's 8th ALU node: out = Src1 * Q(Src0^2).
    def _refP(in0, in1, c0, c1, c2):
        s = in0 * in0
        q = np.float32(1.0) + s * (c0 + s * (c1 + s * c2))
        return in1 * q

    _sP = Src0 * Src0
    _qP = One + _sP * (C0 + _sP * (C1 + _sP * C2))
    bodyP = Src1 * _qP

    _CUSTOM_OPS["A"] = _mk("MISH_RECIP_A", bodyA, _refA)
    _CUSTOM_OPS["B"] = _mk("MISH_RECIP_B", bodyB, _refB)
    _CUSTOM_OPS["M"] = _mk("MISH_MUL_BIAS", bodyM, _refM)
    _CUSTOM_OPS["T"] = _mk("MISH_T2_FAST", bodyT, _refT)
    _CUSTOM_OPS["P"] = _mk("MISH_POLY_MUL", bodyP, _refP)
    return _CUSTOM_OPS


SEED_C0 = -0.23549792
SEED_C1 = 2.0017324
T2_C0 = 1.7340084390144614
T2_C1 = -0.7434944237918216

# minimax cubics for t = (1-s)/(1+s), s = sigmoid(-x)^2 (see _refP):
# P1 on s in [0, 1] (first mish), P2 on s in [0, 0.334] (second mish,
# bounded because mish1 output >= -0.3125).
P1_C = (-1.924537766665479, 1.4214524363965337, -0.4999994122815388)
P2_C = (-1.9928074644613645, 1.8571218730908667, -1.1389181239973036)


def build_nc(
    identity_act=False,
    xw_bufs=2,
    pg_bufs=4,
    mid_bufs=3,
    out_bufs=2,
    skip_dma_out=False,
    yb_mode="split",
    NB=2,
):
    """Build the single-core Bass program (SPMD across 8 cores).

    Per-group elementwise chain (group = 4 PSUM banks = 2032 outputs/part):
      q1 = sigmoid(-(z+b))   [ACT, reads PSUM]
      yb = z + b             [Pool tensor_scalar (or ACT / ACT+DVE split)]
      p  = yb * Q3(q1^2)     [DVE custom P1; Q3 ~ (1-s)/(1+s), so p = mish1]
      q2 = sigmoid(-p)       [ACT]
      out= p * Q3'(q2^2)     [DVE custom P2 -> strided outt = mish2]
    The q2/P2 tail of group g is emitted after the q1/yb head of group g+1
    so each engine's queue stays in dataflow order and PSUM frees early.
    """
    ops = _register_custom_ops()
    OPP = ops["P"]
    nc = bacc.Bacc("TRN2", target_bir_lowering=False, debug=False, num_devices=1)

    x_d = nc.dram_tensor("x", [CIN, H, W], F32R, kind="ExternalInput")
    wpe_d = nc.dram_tensor("wpe", [128, KHW, COUT], F32R, kind="ExternalInput")
    wpo_d = nc.dram_tensor("wpo", [128, KHW, COUT], F32R, kind="ExternalInput")
    wse_d = nc.dram_tensor("wse", [64, KHW, COUT], F32R, kind="ExternalInput")
    wso_d = nc.dram_tensor("wso", [64, KHW, COUT], F32R, kind="ExternalInput")
    bias_d = nc.dram_tensor("bias", [COUT, 1], F32, kind="ExternalInput")
    y_d = nc.dram_tensor("y", [COUT, HO, WO], BF16, kind="ExternalOutput")

    y_ap = y_d.ap()

    with tile.TileContext(nc) as tc:
        with (
            tc.tile_pool(name="wpool", bufs=1) as wpool,
            tc.tile_pool(name="xpool", bufs=xw_bufs) as xpool,
            tc.tile_pool(name="ppool", bufs=pg_bufs, space="PSUM") as ppool,
            tc.tile_pool(name="mpool", bufs=mid_bufs) as mpool,
            tc.tile_pool(name="opool", bufs=out_bufs) as opool,
        ):
            # ---- constants ----
            wpe = wpool.tile([128, KHW, COUT], F32R, tag="wpe")
            wpo = wpool.tile([128, KHW, COUT], F32R, tag="wpo")
            wse = wpool.tile([64, KHW, COUT], F32R, tag="wse")
            wso = wpool.tile([128, KHW, COUT], F32R, tag="wso")  # data in parts 64:128
            bias = wpool.tile([COUT, 1], F32, tag="bias")
            nbias = wpool.tile([COUT, 1], F32, tag="nbias")

            stages = []  # software-pipeline state, one entry per group

            def load_chunk(ro0, split=None):
                # load chunk: input rows ro0 .. ro0+33, parity-split.
                # split=k loads row-pairs [0,k) first (so the first groups'
                # matmuls can start before the whole chunk arrives).
                xw = xpool.tile([128, NI, W], F32R, tag="xw")
                src = x_d.ap()[:, ro0 : ro0 + 2 * NI, :].rearrange(
                    "c (i two) w -> two c i w", two=2
                )
                pieces = [(0, NI)] if split is None else [(0, split), (split, NI)]
                for i0, i1 in pieces:
                    nc.sync.dma_start(xw[0:64, i0:i1, :], src[0][:, i0:i1, :])
                    nc.sync.dma_start(xw[64:128, i0:i1, :], src[1][:, i0:i1, :])
                return xw

            # wpe + the first slice of chunk 0 come first so the PE can
            # start within ~4us; everything else loads behind them.
            nc.sync.dma_start(wpe[:], wpe_d.ap())
            xw = load_chunk(CHUNKS[0], split=6)
            nc.sync.dma_start(wpo[:], wpo_d.ap())
            nc.sync.dma_start(wse[:], wse_d.ap())
            nc.sync.dma_start(wso[64:128, :, :], wso_d.ap())
            nc.sync.dma_start(bias[:], bias_d.ap())
            nc.vector.tensor_scalar_mul(nbias[:], bias[:], -1.0)
            R = 4 * NB  # output rows per (lo0, both-par) pair = outt rows
            warm = [True]
            for ci, ro0 in enumerate(CHUNKS):
                for lo0 in range(0, 32, R):
                    for par in (0, 1):  # even rows then odd rows
                        pg = ppool.tile([128, NB, 512], F32, tag="pg")
                        if warm[0]:
                            # ramp the PE p-state cheaply: the first ~36
                            # queued instructions are costed at the cold
                            # clock, so make them tiny (80-col) matmuls
                            # that still span >3us of engine time.
                            warm[0] = False
                            for _ in range(34):
                                nc.tensor.matmul(
                                    pg[:, 0, 0:32],
                                    wpe[:, 0, :],
                                    xw[:, 0:1, 0:32],
                                    start=True,
                                    stop=True,
                                )
                        for b in range(NB):  # NB blocks of 2 rows (stride 2)
                            lr = lo0 + par + 4 * b  # first output row of block
                            i = (lr - par) // 2  # row-pair index
                            # pair taps: even par: (kh0 lower, kh1 upper) @ i
                            #            odd  par: (kh1 lower, kh2 upper) @ i+1
                            ip = i if par == 0 else i + 1
                            # single tap: even par: kh2, lower @ i+1
                            #             odd  par: kh0, upper @ i
                            is_ = i + 1 if par == 0 else i
                            wp = wpe if par == 0 else wpo
                            for kw in range(KHW):
                                nc.tensor.matmul(
                                    pg[:, b, 0:508],
                                    wp[:, kw, :],
                                    xw[:, ip : ip + 2, kw : kw + 254],
                                    start=(kw == 0),
                                    stop=False,
                                )
                            for kw in range(KHW):
                                if par == 0:
                                    lhsT = wse[:, kw, :]
                                    rhs = xw[0:64, is_ : is_ + 2, kw : kw + 254]
                                else:
                                    lhsT = wso[64:128, kw, :]
                                    rhs = xw[64:128, is_ : is_ + 2, kw : kw + 254]
                                nc.tensor.matmul(
                                    pg[:, b, 0:508],
                                    lhsT,
                                    rhs,
                                    start=False,
                                    stop=(kw == KHW - 1),
                                )
                        if par == 0:
                            outt = opool.tile([128, R * WO], BF16, tag="outt")
                        EW = NB * 508
                        pg_in = pg[:, :, 0:508]
                        dst = outt[:].rearrange(
                            "p (s2 t w) -> p s2 t w", t=2, w=WO
                        )[:, :, par, :]
                        if identity_act:
                            # debug path: conv+bias only
                            nc.scalar.activation(
                                dst, pg_in, AFT.Identity, bias=bias[:]
                            )
                            continue
                        # ---- head of this group's elementwise chain ----
                        # yb = z+b is the only reader of this PSUM buffer,
                        # so the buffer frees early; q1 reads yb from SBUF.
                        # Bank 0 goes through ACT (identity+bias); bank 1
                        # is DMA-copied to SBUF and biased on Pool, keeping
                        # ACT under the PE group budget.
                        yb = mpool.tile([128, EW], F32, tag="yb")
                        if yb_mode == "act":
                            nc.scalar.activation(
                                yb[:], pg_in, AFT.Identity, bias=bias[:]
                            )
                        elif yb_mode == "dmapool":
                            ban = NB - 1
                            nc.scalar.activation(
                                yb[:, 0 : ban * 508],
                                pg[:, 0:ban, 0:508],
                                AFT.Identity,
                                bias=bias[:],
                            )
                            ybr = mpool.tile([128, (NB - ban) * 508], F32,
                                             tag="ybr")
                            nc.sync.dma_start(ybr[:], pg[:, ban:NB, 0:508])
                            nc.gpsimd.tensor_scalar_add(
                                yb[:, ban * 508 : EW], ybr[:], bias[:]
                            )
                        else:  # "split": NB-1 banks on ACT, 1 bank on DVE
                            ban = NB - 1
                            nc.scalar.activation(
                                yb[:, 0 : ban * 508],
                                pg[:, 0:ban, 0:508],
                                AFT.Identity,
                                bias=bias[:],
                            )
                            nc.vector.tensor_scalar_add(
                                yb[:, ban * 508 : EW],
                                pg[:, ban:NB, 0:508],
                                bias[:],
                            )
                        q1 = mpool.tile([128, EW], F32, tag="q")
                        nc.scalar.activation(
                            q1[:], yb[:], AFT.Sigmoid, scale=-1.0
                        )

                        # ---- software pipeline: emit deferred stages so
                        # every cross-engine dependency is >= 1 cycle old
                        # (P1 one cycle late, q2 two, P2+DMA three) ----
                        def st_p1(q1=q1, yb=yb):
                            p = mpool.tile([128, EW], F32, tag="p")
                            nc.vector._custom_dve(
                                OPP, out=p[:], in0=q1[:], in1=yb[:],
                                s0=P1_C[0], s1=P1_C[1], imm2=P1_C[2],
                            )
                            return p

                        def st_q2(p):
                            q2 = mpool.tile([128, EW], F32, tag="q2")
                            nc.scalar.activation(
                                q2[:], p[:], AFT.Sigmoid, scale=-1.0
                            )
                            return q2

                        def st_p2(p, q2, dst=dst, outt=outt,
                                  par=par, ro0=ro0, lo0=lo0):
                            nc.vector._custom_dve(
                                OPP, out=dst, in0=q2[:], in1=p[:],
                                s0=P2_C[0], s1=P2_C[1], imm2=P2_C[2],
                            )
                            if par == 1 and not skip_dma_out:
                                r0 = ro0 + lo0
                                nc.sync.dma_start(
                                    y_ap[:, r0 : r0 + R, :], outt[:]
                                )

                        stages.append({"p1": st_p1, "q2": st_q2, "p2": st_p2})

                        def advance():
                            # run stage k of the group k cycles back
                            n = len(stages)
                            if n >= 2 and "p" not in stages[-2]:
                                g = stages[-2]
                                g["p"] = g["p1"]()
                            if n >= 3 and "q2v" not in stages[-3]:
                                g = stages[-3]
                                g["q2v"] = g["q2"](g["p"])
                            if n >= 4 and "done" not in stages[-4]:
                                g = stages[-4]
                                g["p2"](g["p"], g["q2v"])
                                g["done"] = True

                        advance()
                        if lo0 == 0 and par == 0 and ci + 1 < len(CHUNKS):
                            # prefetch next chunk's input while this chunk
                            # computes (ahead of this chunk's out-DMAs in
                            # the queue, so PE never waits at the boundary)
                            xw_next = load_chunk(CHUNKS[ci + 1])
                xw = xw_next
            # drain the pipeline (3 more virtual cycles)
            for _ in range(3):
                stages.append({})
                n = len(stages)
                if "p1" in stages[-2] and "p" not in stages[-2]:
                    g = stages[-2]
                    g["p"] = g["p1"]()
                if n >= 3 and "q2" in stages[-3] and "q2v" not in stages[-3]:
                    g = stages[-3]
                    g["q2v"] = g["q2"](g["p"])
                if n >= 4 and "p2" in stages[-4] and "done" not in stages[-4]:
                    g = stages[-4]
                    g["p2"](g["p"], g["q2v"])
                    g["done"] = True

    nc.compile()
    return nc


def pack_inputs(x, weight, bias_v):
    """Host-side packing: fp32r rounding + weight tap stacking + per-core x."""
    x = np.ascontiguousarray(np.asarray(x, dtype=np.float32))
    weight = np.ascontiguousarray(np.asarray(weight, dtype=np.float32))
    bias_v = np.ascontiguousarray(np.asarray(bias_v, dtype=np.float32))

    wr = fp32r_round(weight)  # [cout, cin, kh, kw]
    wT = wr.transpose(1, 0, 2, 3)  # [cin, cout, kh, kw]

    def lhsT(kh):  # [cin, kw, cout] -> slice per kw gives [cin, cout]
        return np.ascontiguousarray(wT[:, :, kh, :].transpose(0, 2, 1))

    k0, k1, k2 = lhsT(0), lhsT(1), lhsT(2)
    wpe = np.concatenate([k0, k1], axis=0)  # even pairs: kh0 lower, kh1 upper
    wpo = np.concatenate([k1, k2], axis=0)  # odd pairs:  kh1 lower, kh2 upper
    wse = k2  # even single: kh2, lower
    wso = k0  # odd single:  kh0, upper

    xr = fp32r_round(x)
    common = {
        "wpe": wpe,
        "wpo": wpo,
        "wse": wse,
        "wso": wso,
        "bias": bias_v.reshape(COUT, 1),
    }
    in_maps = [
        dict(common, x=np.ascontiguousarray(xr[n])) for n in range(xr.shape[0])
    ]
    return in_maps


_NC_CACHE = {}


def _get_nc():
    if "nc" not in _NC_CACHE:
        _NC_CACHE["nc"] = build_nc()
    return _NC_CACHE["nc"]


def kernel(x, weight, bias):
    nc = _get_nc()
    in_maps = pack_inputs(x, weight, bias)
    res = run_bass_kernel_spmd(nc, in_maps, core_ids=list(range(NCORES)))
    y = np.stack(
        [
            np.asarray(res.results[n]["y"]).astype(np.float32)
            for n in range(NCORES)
        ],
        axis=0,
    )
    return y



# revision 48
# speedup vs baseline: 1.0106x; 1.0106x over previous
"""Trainium2 Bass kernel: conv2d(64->128, 3x3, valid) + bias + mish(mish(.)).

Full inputs:  x [8, 64, 256, 256] f32, weight [128, 64, 3, 3] f32, bias [128] f32
Full output:  y [8, 128, 254, 254] f32

Sharding: data-parallel over batch, image n -> NeuronCore n (8 cores).

Per-core strategy:
  * SBUF x layout is parity-split: partitions 0-63 hold (cin, even rows),
    partitions 64-127 hold (cin, odd rows), both as [cin, i, col] with the
    same free offset for row pair (2i, 2i+1).  A 3x3 conv tap pair
    (kh, kh+1) then contracts over all 128 partitions in ONE matmul, and
    the leftover tap is a 64-deep matmul, so each 2-row output block is
    6 matmuls (3 pair + 3 single) of free size 2x254=508 accumulated in
    one PSUM bank: 4.5 "full" matmuls of work in 6 instructions.
  * Matmuls run in float32r (fp32 with mantissa rounded to 11 bits; inputs
    pre-rounded on host) which streams at bf16 rate for free dims >= 256.
    Groups of 2 PSUM banks with a 4-deep PSUM pipeline keep the PE gaps
    short, which keeps the tensor engine in its fast p-state; a burst of
    tiny warm-up matmuls ramps the clock before the first real ones.
  * mish(mish(z+b)) with z the conv output: per mish, q = sigmoid(-x) on
    ScalarE (real HW table), then ONE fused 8-node custom DVE op computes
    x * Q3(q^2) where Q3 is a minimax cubic of (1-s)/(1+s) on the proper
    range (~3e-3 / ~1e-4 abs err), i.e. x*tanh(softplus(x)).  So the whole
    elementwise tail is 5 passes: q1 [ACT], z+b copy [ACT for bank 0; the
    PE pre-loads bias into bank 1's PSUM via a contract-1 matmul so that
    bank needs no copy], P1 [DVE], q2 [ACT], P2 [DVE, writes bf16 output
    tile directly].  Stage emission is software-pipelined (P1 one group
    late, q2 two, P2+out-DMA three) so cross-engine deps span >=1 group.
  * Output is written/DMA'd as bf16 (cast to f32 on host) to halve the
    output DMA traffic; rel err stays ~3e-3, far under the 2e-2 gate.
  * Output rows processed in 8 chunks of 32 (last chunk overlaps 2 rows so
    every chunk/block/group has identical shape); next chunk's input DMA
    is prefetched from behind one compute group so the PE never waits at
    chunk boundaries.
"""

import sys

sys.path.insert(0, "/opt/trn_rl_repo")

import numpy as np

import concourse.bass as bass
import concourse.mybir as mybir
import concourse.tile as tile
from concourse import bacc
from concourse.bass_utils import run_bass_kernel_spmd

F32 = mybir.dt.float32
F32R = mybir.dt.float32r
BF16 = mybir.dt.bfloat16
AFT = mybir.ActivationFunctionType

CIN, H, W = 64, 256, 256
COUT, KHW = 128, 3
HO, WO = 254, 254
NCORES = 8
NI = 17  # row-pairs held in SBUF per chunk (34 input rows)

# chunk starts; every chunk computes 32 output rows (last overlaps by 2)
CHUNKS = [0, 32, 64, 96, 128, 160, 192, 222]


def _patch_act_tables():
    """concourse's act-table map drops functions living in the generic 'act2'
    pwp slot, so Mish (present on TRN2 in the mish_and_others set) looks
    unavailable.  Re-add it for this process."""
    import concourse.hw_specs as hw_specs
    import concourse.bacc as bacc_mod

    if getattr(bacc_mod, "_mish_patch", False):
        return
    orig = hw_specs.get_activation_tables

    def patched(module_arch):
        t = dict(orig(module_arch))
        if "mish_and_others" in t:
            t["mish_and_others"] = set(t["mish_and_others"]) | {AFT.Mish}
        return t

    bacc_mod.get_activation_tables = patched
    bacc_mod._mish_patch = True


def fp32r_round(a: np.ndarray) -> np.ndarray:
    """Round fp32 to the fp32r grid (mantissa to 11 explicit bits, RNE).
    Matches walrus's fp32_to_fp32r bit-exactly for finite values."""
    a = np.ascontiguousarray(a, dtype=np.float32)
    b = a.view(np.uint32)
    r = (b + np.uint32(0x7FF) + ((b >> np.uint32(12)) & np.uint32(1))) & np.uint32(
        0xFFFFF000
    )
    return r.view(np.float32)


_CUSTOM_OPS = {}


def _register_custom_ops():
    """Register two custom DVE ops used for mish:
      MISH_RECIP_A(q)      = y1 ~ 1/(1+q^2)   (NOT-seed + 1 NR)
      MISH_RECIP_B(q, y1)  = 1 - 2*y2, y2 = NR(y1)  == -(1-q^2)/(1+q^2) = -tanh(softplus)
    mish(x) = x*t with t = (1-q^2)/(1+q^2), q = sigmoid(-x).
    """
    if _CUSTOM_OPS:
        return _CUSTOM_OPS
    import re as _re

    import concourse.dve_ops as dv
    from concourse.dve_spec import AluOp, Bin, Spec, Src0, Src1, C0, C1, C2

    from concourse.dve_spec import One

    def _refA(in0, in1, c0, c1, c2):
        d = in0 * in0 + np.float32(1.0)
        n = (~d.view(np.int32)).view(np.float32)
        y0 = n * c0
        return y0 * (c1 - d * y0)

    def _refB(in0, in1, c0, c1, c2):
        d = in0 * in0 + np.float32(1.0)
        y2 = in1 * (c0 - d * in1)
        return (np.float32(1.0) - in0 * in0) * y2

    def _refM(in0, in1, c0, c1, c2):
        in0 = in0.reshape(in0.shape[0], -1)
        in1 = in1.reshape(in1.shape[0], -1)
        if isinstance(c0, np.ndarray):
            c0 = c0.reshape(c0.shape[0], 1)
        return (in0 + c0) * in1

    _dA = Src0 * Src0 + One
    _nA = Bin(AluOp.BITWISE_NOT, _dA, _dA)
    _y0 = _nA * C0
    bodyA = _y0 * (C1 - _dA * _y0)

    _dB = Src0 * Src0 + One
    _y2 = Src1 * (C0 - _dB * Src1)
    bodyB = (One - Src0 * Src0) * _y2

    bodyM = (Src0 + C0) * Src1

    def _mk(name, body, ref):
        spec = Spec(body=body, reference=ref)
        op = dv.DveOp(name, spec, subdim=False, uops_sha={})
        # register row so compile() can resolve the opcode
        if name not in dv._SUB_OPCODE_FOR_NAME:
            dv._SUB_OPCODE_FOR_NAME[name] = max(dv._SUB_OPCODE_FOR_NAME.values()) + 1
        try:
            op.compile("v3")
        except ValueError as e:
            m = _re.search(r'uops_sha\["v3"\]="([0-9a-f]+)"', str(e))
            assert m, f"no sha in: {e}"
            op = dv.DveOp(name, spec, subdim=False, uops_sha={"v3": m.group(1)})
        op.compile("v3")
        dv.OPS.append(op)
        dv.CUSTOM_DVE_SPECS[name] = spec
        return op

    # T2: from s = q^2 (bounded [0, 0.345] because mish >= -0.309):
    # d = s+1; y0 = C0 + C1*d (linear minimax seed of 1/d on [1,1.345]);
    # y1 = y0*(C2 - d*y0) (one Newton step, C2=2); out = (1-s)*y1 = t2.
    def _refT(in0, in1, c0, c1, c2):
        d = in0 + np.float32(1.0)
        y0 = c0 + c1 * d
        y1 = y0 * (c2 - d * y0)
        return (np.float32(1.0) - in0) * y1

    _dT = Src0 + One
    _y0T = C0 + C1 * _dT
    _y1T = _y0T * (C2 - _dT * _y0T)
    bodyT = (One - Src0) * _y1T

    # P1/P2: one fused op per mish.  t = (1-s)/(1+s) with s = q^2 is
    # approximated by the minimax cubic Q(s) = 1 + s*(C0 + s*(C1 + s*C2))
    # (constant term pinned at 1), and the product with the pre-activation
    # is folded into the op's 8th ALU node: out = Src1 * Q(Src0^2).
    def _refP(in0, in1, c0, c1, c2):
        s = in0 * in0
        q = np.float32(1.0) + s * (c0 + s * (c1 + s * c2))
        return in1 * q

    _sP = Src0 * Src0
    _qP = One + _sP * (C0 + _sP * (C1 + _sP * C2))
    bodyP = Src1 * _qP

    _CUSTOM_OPS["A"] = _mk("MISH_RECIP_A", bodyA, _refA)
    _CUSTOM_OPS["B"] = _mk("MISH_RECIP_B", bodyB, _refB)
    _CUSTOM_OPS["M"] = _mk("MISH_MUL_BIAS", bodyM, _refM)
    _CUSTOM_OPS["T"] = _mk("MISH_T2_FAST", bodyT, _refT)
    _CUSTOM_OPS["P"] = _mk("MISH_POLY_MUL", bodyP, _refP)
    return _CUSTOM_OPS


SEED_C0 = -0.23549792
SEED_C1 = 2.0017324
T2_C0 = 1.7340084390144614
T2_C1 = -0.7434944237918216

# minimax cubics for t = (1-s)/(1+s), s = sigmoid(-x)^2 (see _refP):
# P1 on s in [0, 1] (first mish), P2 on s in [0, 0.334] (second mish,
# bounded because mish1 output >= -0.3125).
P1_C = (-1.924537766665479, 1.4214524363965337, -0.4999994122815388)
P2_C = (-1.9928074644613645, 1.8571218730908667, -1.1389181239973036)


def build_nc(
    identity_act=False,
    xw_bufs=2,
    pg_bufs=4,
    mid_bufs=3,
    out_bufs=2,
    skip_dma_out=False,
    yb_mode="biasmm",
    NB=2,
):
    """Build the single-core Bass program (SPMD across 8 cores).

    Per-group elementwise chain (group = 4 PSUM banks = 2032 outputs/part):
      q1 = sigmoid(-(z+b))   [ACT, reads PSUM]
      yb = z + b             [Pool tensor_scalar (or ACT / ACT+DVE split)]
      p  = yb * Q3(q1^2)     [DVE custom P1; Q3 ~ (1-s)/(1+s), so p = mish1]
      q2 = sigmoid(-p)       [ACT]
      out= p * Q3'(q2^2)     [DVE custom P2 -> strided outt = mish2]
    The q2/P2 tail of group g is emitted after the q1/yb head of group g+1
    so each engine's queue stays in dataflow order and PSUM frees early.
    """
    ops = _register_custom_ops()
    OPP = ops["P"]
    nc = bacc.Bacc("TRN2", target_bir_lowering=False, debug=False, num_devices=1)

    x_d = nc.dram_tensor("x", [CIN, H, W], F32R, kind="ExternalInput")
    wpe_d = nc.dram_tensor("wpe", [128, KHW, COUT], F32R, kind="ExternalInput")
    wpo_d = nc.dram_tensor("wpo", [128, KHW, COUT], F32R, kind="ExternalInput")
    wse_d = nc.dram_tensor("wse", [64, KHW, COUT], F32R, kind="ExternalInput")
    wso_d = nc.dram_tensor("wso", [64, KHW, COUT], F32R, kind="ExternalInput")
    bias_d = nc.dram_tensor("bias", [COUT, 1], F32, kind="ExternalInput")
    biasrow_d = nc.dram_tensor("biasrow", [1, COUT], F32R, kind="ExternalInput")
    onesrow_d = nc.dram_tensor("onesrow", [1, 512], F32R, kind="ExternalInput")
    y_d = nc.dram_tensor("y", [COUT, HO, WO], BF16, kind="ExternalOutput")

    y_ap = y_d.ap()

    with tile.TileContext(nc) as tc:
        with (
            tc.tile_pool(name="wpool", bufs=1) as wpool,
            tc.tile_pool(name="xpool", bufs=xw_bufs) as xpool,
            tc.tile_pool(name="ppool", bufs=pg_bufs, space="PSUM") as ppool,
            tc.tile_pool(name="mpool", bufs=mid_bufs) as mpool,
            tc.tile_pool(name="opool", bufs=out_bufs) as opool,
        ):
            # ---- constants ----
            wpe = wpool.tile([128, KHW, COUT], F32R, tag="wpe")
            wpo = wpool.tile([128, KHW, COUT], F32R, tag="wpo")
            wse = wpool.tile([64, KHW, COUT], F32R, tag="wse")
            wso = wpool.tile([128, KHW, COUT], F32R, tag="wso")  # data in parts 64:128
            bias = wpool.tile([COUT, 1], F32, tag="bias")
            biasrow = wpool.tile([1, COUT], F32R, tag="biasrow")
            ones = wpool.tile([1, 512], F32R, tag="ones")
            nbias = wpool.tile([COUT, 1], F32, tag="nbias")

            stages = []  # software-pipeline state, one entry per group

            def load_chunk(ro0, split=None):
                # load chunk: input rows ro0 .. ro0+33, parity-split.
                # split=k loads row-pairs [0,k) first (so the first groups'
                # matmuls can start before the whole chunk arrives).
                xw = xpool.tile([128, NI, W], F32R, tag="xw")
                src = x_d.ap()[:, ro0 : ro0 + 2 * NI, :].rearrange(
                    "c (i two) w -> two c i w", two=2
                )
                pieces = [(0, NI)] if split is None else [(0, split), (split, NI)]
                for i0, i1 in pieces:
                    nc.sync.dma_start(xw[0:64, i0:i1, :], src[0][:, i0:i1, :])
                    nc.sync.dma_start(xw[64:128, i0:i1, :], src[1][:, i0:i1, :])
                return xw

            # wpe + the first slice of chunk 0 come first so the PE can
            # start within ~4us; everything else loads behind them.
            nc.sync.dma_start(wpe[:], wpe_d.ap())
            xw = load_chunk(CHUNKS[0], split=6)
            nc.sync.dma_start(wpo[:], wpo_d.ap())
            nc.sync.dma_start(wse[:], wse_d.ap())
            nc.sync.dma_start(wso[64:128, :, :], wso_d.ap())
            nc.sync.dma_start(bias[:], bias_d.ap())
            nc.sync.dma_start(biasrow[:], biasrow_d.ap())
            nc.sync.dma_start(ones[:], onesrow_d.ap())
            nc.vector.tensor_scalar_mul(nbias[:], bias[:], -1.0)
            R = 4 * NB  # output rows per (lo0, both-par) pair = outt rows
            warm = [True]
            for ci, ro0 in enumerate(CHUNKS):
                for lo0 in range(0, 32, R):
                    for par in (0, 1):  # even rows then odd rows
                        pg = ppool.tile([128, NB, 512], F32, tag="pg")
                        if warm[0]:
                            # ramp the PE p-state cheaply: the first ~36
                            # queued instructions are costed at the cold
                            # clock, so make them tiny (80-col) matmuls
                            # that still span >3us of engine time.
                            warm[0] = False
                            for _ in range(34):
                                nc.tensor.matmul(
                                    pg[:, 0, 0:32],
                                    wpe[:, 0, :],
                                    xw[:, 0:1, 0:32],
                                    start=True,
                                    stop=True,
                                )
                        for b in range(NB):  # NB blocks of 2 rows (stride 2)
                            if yb_mode == "biasmm" and b == NB - 1:
                                # bank NB-1: pre-load the bias into PSUM
                                # via a contract-1 matmul (bias row x ones)
                                # so this bank's chain reads (z+b) straight
                                # from PSUM and needs no yb copy pass.
                                nc.tensor.matmul(
                                    pg[:, b, 0:508],
                                    biasrow[0:1, :],
                                    ones[0:1, 0:508],
                                    start=True,
                                    stop=False,
                                )
                            lr = lo0 + par + 4 * b  # first output row of block
                            i = (lr - par) // 2  # row-pair index
                            # pair taps: even par: (kh0 lower, kh1 upper) @ i
                            #            odd  par: (kh1 lower, kh2 upper) @ i+1
                            ip = i if par == 0 else i + 1
                            # single tap: even par: kh2, lower @ i+1
                            #             odd  par: kh0, upper @ i
                            is_ = i + 1 if par == 0 else i
                            wp = wpe if par == 0 else wpo
                            bias_in_psum = yb_mode == "biasmm" and b == NB - 1
                            for kw in range(KHW):
                                nc.tensor.matmul(
                                    pg[:, b, 0:508],
                                    wp[:, kw, :],
                                    xw[:, ip : ip + 2, kw : kw + 254],
                                    start=(kw == 0 and not bias_in_psum),
                                    stop=False,
                                )
                            for kw in range(KHW):
                                if par == 0:
                                    lhsT = wse[:, kw, :]
                                    rhs = xw[0:64, is_ : is_ + 2, kw : kw + 254]
                                else:
                                    lhsT = wso[64:128, kw, :]
                                    rhs = xw[64:128, is_ : is_ + 2, kw : kw + 254]
                                nc.tensor.matmul(
                                    pg[:, b, 0:508],
                                    lhsT,
                                    rhs,
                                    start=False,
                                    stop=(kw == KHW - 1),
                                )
                        if par == 0:
                            outt = opool.tile([128, R * WO], BF16, tag="outt")
                        EW = NB * 508
                        pg_in = pg[:, :, 0:508]
                        dst = outt[:].rearrange(
                            "p (s2 t w) -> p s2 t w", t=2, w=WO
                        )[:, :, par, :]
                        if identity_act:
                            # debug path: conv+bias only
                            nc.scalar.activation(
                                dst, pg_in, AFT.Identity, bias=bias[:]
                            )
                            continue
                        # ---- head of this group's elementwise chain ----
                        # yb = z+b for bank 0 via ACT identity (the only
                        # PSUM reader for that bank, so it frees early).
                        # In biasmm mode the last bank already holds z+b in
                        # PSUM (bias pre-loaded by the PE), so q1/P1 read it
                        # directly and no copy pass is needed.
                        ban = NB - 1 if yb_mode == "biasmm" else NB
                        yb = mpool.tile([128, ban * 508], F32, tag="yb")
                        q1 = mpool.tile([128, EW], F32, tag="q")
                        if yb_mode == "split":
                            nc.scalar.activation(
                                yb[:, 0 : (NB - 1) * 508],
                                pg[:, 0 : NB - 1, 0:508],
                                AFT.Identity,
                                bias=bias[:],
                            )
                            nc.vector.tensor_scalar_add(
                                yb[:, (NB - 1) * 508 :],
                                pg[:, NB - 1 : NB, 0:508],
                                bias[:],
                            )
                            nc.scalar.activation(
                                q1[:], yb[:], AFT.Sigmoid, scale=-1.0
                            )
                        elif yb_mode == "act":
                            nc.scalar.activation(
                                yb[:], pg_in, AFT.Identity, bias=bias[:]
                            )
                            nc.scalar.activation(
                                q1[:], yb[:], AFT.Sigmoid, scale=-1.0
                            )
                        else:  # "biasmm"
                            nc.scalar.activation(
                                yb[:], pg[:, 0:ban, 0:508],
                                AFT.Identity, bias=bias[:],
                            )
                            nc.scalar.activation(
                                q1[:, 0 : ban * 508], yb[:],
                                AFT.Sigmoid, scale=-1.0,
                            )
                            nc.scalar.activation(
                                q1[:, ban * 508 : EW],
                                pg[:, ban:NB, 0:508],
                                AFT.Sigmoid, scale=-1.0,
                            )

                        # ---- software pipeline: emit deferred stages so
                        # every cross-engine dependency is >= 1 cycle old
                        # (P1 one cycle late, q2 two, P2+DMA three) ----
                        def st_p1(q1=q1, yb=yb, pg=pg, ban=ban):
                            p = mpool.tile([128, EW], F32, tag="p")
                            nc.vector._custom_dve(
                                OPP,
                                out=p[:, 0 : ban * 508],
                                in0=q1[:, 0 : ban * 508],
                                in1=yb[:],
                                s0=P1_C[0], s1=P1_C[1], imm2=P1_C[2],
                            )
                            if ban < NB:
                                nc.vector._custom_dve(
                                    OPP,
                                    out=p[:, ban * 508 : EW],
                                    in0=q1[:, ban * 508 : EW],
                                    in1=pg[:, ban, 0:508],
                                    s0=P1_C[0], s1=P1_C[1], imm2=P1_C[2],
                                )
                            return p

                        def st_q2(p):
                            q2 = mpool.tile([128, EW], F32, tag="q2")
                            nc.scalar.activation(
                                q2[:], p[:], AFT.Sigmoid, scale=-1.0
                            )
                            return q2

                        def st_p2(p, q2, dst=dst, outt=outt,
                                  par=par, ro0=ro0, lo0=lo0):
                            nc.vector._custom_dve(
                                OPP, out=dst, in0=q2[:], in1=p[:],
                                s0=P2_C[0], s1=P2_C[1], imm2=P2_C[2],
                            )
                            if par == 1 and not skip_dma_out:
                                r0 = ro0 + lo0
                                nc.sync.dma_start(
                                    y_ap[:, r0 : r0 + R, :], outt[:]
                                )

                        stages.append({"p1": st_p1, "q2": st_q2, "p2": st_p2})

                        def advance():
                            # run stage k of the group k cycles back
                            n = len(stages)
                            if n >= 2 and "p" not in stages[-2]:
                                g = stages[-2]
                                g["p"] = g["p1"]()
                            if n >= 3 and "q2v" not in stages[-3]:
                                g = stages[-3]
                                g["q2v"] = g["q2"](g["p"])
                            if n >= 4 and "done" not in stages[-4]:
                                g = stages[-4]
                                g["p2"](g["p"], g["q2v"])
                                g["done"] = True

                        advance()
                        if lo0 == 0 and par == 0 and ci + 1 < len(CHUNKS):
                            # prefetch next chunk's input while this chunk
                            # computes (ahead of this chunk's out-DMAs in
                            # the queue, so PE never waits at the boundary)
                            xw_next = load_chunk(CHUNKS[ci + 1])
                xw = xw_next
            # drain the pipeline (3 more virtual cycles)
            for _ in range(3):
                stages.append({})
                n = len(stages)
                if "p1" in stages[-2] and "p" not in stages[-2]:
                    g = stages[-2]
                    g["p"] = g["p1"]()
                if n >= 3 and "q2" in stages[-3] and "q2v" not in stages[-3]:
                    g = stages[-3]
                    g["q2v"] = g["q2"](g["p"])
                if n >= 4 and "p2" in stages[-4] and "done" not in stages[-4]:
                    g = stages[-4]
                    g["p2"](g["p"], g["q2v"])
                    g["done"] = True

    nc.compile()
    return nc


def pack_inputs(x, weight, bias_v):
    """Host-side packing: fp32r rounding + weight tap stacking + per-core x."""
    x = np.ascontiguousarray(np.asarray(x, dtype=np.float32))
    weight = np.ascontiguousarray(np.asarray(weight, dtype=np.float32))
    bias_v = np.ascontiguousarray(np.asarray(bias_v, dtype=np.float32))

    wr = fp32r_round(weight)  # [cout, cin, kh, kw]
    wT = wr.transpose(1, 0, 2, 3)  # [cin, cout, kh, kw]

    def lhsT(kh):  # [cin, kw, cout] -> slice per kw gives [cin, cout]
        return np.ascontiguousarray(wT[:, :, kh, :].transpose(0, 2, 1))

    k0, k1, k2 = lhsT(0), lhsT(1), lhsT(2)
    wpe = np.concatenate([k0, k1], axis=0)  # even pairs: kh0 lower, kh1 upper
    wpo = np.concatenate([k1, k2], axis=0)  # odd pairs:  kh1 lower, kh2 upper
    wse = k2  # even single: kh2, lower
    wso = k0  # odd single:  kh0, upper

    xr = fp32r_round(x)
    common = {
        "wpe": wpe,
        "wpo": wpo,
        "wse": wse,
        "wso": wso,
        "bias": bias_v.reshape(COUT, 1),
        "biasrow": fp32r_round(bias_v.reshape(1, COUT)),
        "onesrow": np.ones((1, 512), dtype=np.float32),
    }
    in_maps = [
        dict(common, x=np.ascontiguousarray(xr[n])) for n in range(xr.shape[0])
    ]
    return in_maps


_NC_CACHE = {}


def _get_nc():
    if "nc" not in _NC_CACHE:
        _NC_CACHE["nc"] = build_nc()
    return _NC_CACHE["nc"]


def kernel(x, weight, bias):
    nc = _get_nc()
    in_maps = pack_inputs(x, weight, bias)
    res = run_bass_kernel_spmd(nc, in_maps, core_ids=list(range(NCORES)))
    y = np.stack(
        [
            np.asarray(res.results[n]["y"]).astype(np.float32)
            for n in range(NCORES)
        ],
        axis=0,
    )
    return y



# revision 58
# speedup vs baseline: 1.0123x; 1.0018x over previous
"""Trainium2 Bass kernel: conv2d(64->128, 3x3, valid) + bias + mish(mish(.)).

Full inputs:  x [8, 64, 256, 256] f32, weight [128, 64, 3, 3] f32, bias [128] f32
Full output:  y [8, 128, 254, 254] f32

Sharding: data-parallel over batch, image n -> NeuronCore n (8 cores).

Per-core strategy:
  * SBUF x layout is parity-split: partitions 0-63 hold (cin, even rows),
    partitions 64-127 hold (cin, odd rows), both as [cin, i, col] with the
    same free offset for row pair (2i, 2i+1).  A 3x3 conv tap pair
    (kh, kh+1) then contracts over all 128 partitions in ONE matmul, and
    the leftover tap is a 64-deep matmul, so each 2-row output block is
    6 matmuls (3 pair + 3 single) of free size 2x254=508 accumulated in
    one PSUM bank: 4.5 "full" matmuls of work in 6 instructions.
  * Matmuls run in float32r (fp32 with mantissa rounded to 11 bits; inputs
    pre-rounded on host) which streams at bf16 rate for free dims >= 256.
    Groups of 2 PSUM banks with a 4-deep PSUM pipeline keep the PE gaps
    short, which keeps the tensor engine in its fast p-state; a burst of
    tiny warm-up matmuls ramps the clock before the first real ones.
  * mish(mish(z+b)) with z the conv output: per mish, q = sigmoid(-x) on
    ScalarE (real HW table), then ONE fused 8-node custom DVE op computes
    x * Q3(q^2) where Q3 is a minimax cubic of (1-s)/(1+s) on the proper
    range (~3e-3 / ~1e-4 abs err), i.e. x*tanh(softplus(x)).  So the whole
    elementwise tail is 5 passes: q1 [ACT], z+b copy [ACT for bank 0; the
    PE pre-loads bias into bank 1's PSUM via a contract-1 matmul so that
    bank needs no copy], P1 [DVE], q2 [ACT], P2 [DVE, writes bf16 output
    tile directly].  Stage emission is software-pipelined (P1 one group
    late, q2 two, P2+out-DMA three) so cross-engine deps span >=1 group.
  * Output is written/DMA'd as bf16 (cast to f32 on host) to halve the
    output DMA traffic; rel err stays ~3e-3, far under the 2e-2 gate.
  * Output rows processed in 8 chunks of 32 (last chunk overlaps 2 rows so
    every chunk/block/group has identical shape); next chunk's input DMA
    is prefetched from behind one compute group so the PE never waits at
    chunk boundaries.
"""

import sys

sys.path.insert(0, "/opt/trn_rl_repo")

import numpy as np

import concourse.bass as bass
import concourse.mybir as mybir
import concourse.tile as tile
from concourse import bacc
from concourse.bass_utils import run_bass_kernel_spmd

F32 = mybir.dt.float32
F32R = mybir.dt.float32r
BF16 = mybir.dt.bfloat16
AFT = mybir.ActivationFunctionType

CIN, H, W = 64, 256, 256
COUT, KHW = 128, 3
HO, WO = 254, 254
NCORES = 8
NI = 17  # row-pairs held in SBUF per chunk (34 input rows)

# chunk starts; every chunk computes 32 output rows (last overlaps by 2)
CHUNKS = [0, 32, 64, 96, 128, 160, 192, 222]


def _patch_act_tables():
    """concourse's act-table map drops functions living in the generic 'act2'
    pwp slot, so Mish (present on TRN2 in the mish_and_others set) looks
    unavailable.  Re-add it for this process."""
    import concourse.hw_specs as hw_specs
    import concourse.bacc as bacc_mod

    if getattr(bacc_mod, "_mish_patch", False):
        return
    orig = hw_specs.get_activation_tables

    def patched(module_arch):
        t = dict(orig(module_arch))
        if "mish_and_others" in t:
            t["mish_and_others"] = set(t["mish_and_others"]) | {AFT.Mish}
        return t

    bacc_mod.get_activation_tables = patched
    bacc_mod._mish_patch = True


def fp32r_round(a: np.ndarray) -> np.ndarray:
    """Round fp32 to the fp32r grid (mantissa to 11 explicit bits, RNE).
    Matches walrus's fp32_to_fp32r bit-exactly for finite values."""
    a = np.ascontiguousarray(a, dtype=np.float32)
    b = a.view(np.uint32)
    r = (b + np.uint32(0x7FF) + ((b >> np.uint32(12)) & np.uint32(1))) & np.uint32(
        0xFFFFF000
    )
    return r.view(np.float32)


_CUSTOM_OPS = {}


def _register_custom_ops():
    """Register two custom DVE ops used for mish:
      MISH_RECIP_A(q)      = y1 ~ 1/(1+q^2)   (NOT-seed + 1 NR)
      MISH_RECIP_B(q, y1)  = 1 - 2*y2, y2 = NR(y1)  == -(1-q^2)/(1+q^2) = -tanh(softplus)
    mish(x) = x*t with t = (1-q^2)/(1+q^2), q = sigmoid(-x).
    """
    if _CUSTOM_OPS:
        return _CUSTOM_OPS
    import re as _re

    import concourse.dve_ops as dv
    from concourse.dve_spec import AluOp, Bin, Spec, Src0, Src1, C0, C1, C2

    from concourse.dve_spec import One

    def _refA(in0, in1, c0, c1, c2):
        d = in0 * in0 + np.float32(1.0)
        n = (~d.view(np.int32)).view(np.float32)
        y0 = n * c0
        return y0 * (c1 - d * y0)

    def _refB(in0, in1, c0, c1, c2):
        d = in0 * in0 + np.float32(1.0)
        y2 = in1 * (c0 - d * in1)
        return (np.float32(1.0) - in0 * in0) * y2

    def _refM(in0, in1, c0, c1, c2):
        in0 = in0.reshape(in0.shape[0], -1)
        in1 = in1.reshape(in1.shape[0], -1)
        if isinstance(c0, np.ndarray):
            c0 = c0.reshape(c0.shape[0], 1)
        return (in0 + c0) * in1

    _dA = Src0 * Src0 + One
    _nA = Bin(AluOp.BITWISE_NOT, _dA, _dA)
    _y0 = _nA * C0
    bodyA = _y0 * (C1 - _dA * _y0)

    _dB = Src0 * Src0 + One
    _y2 = Src1 * (C0 - _dB * Src1)
    bodyB = (One - Src0 * Src0) * _y2

    bodyM = (Src0 + C0) * Src1

    def _mk(name, body, ref):
        spec = Spec(body=body, reference=ref)
        op = dv.DveOp(name, spec, subdim=False, uops_sha={})
        # register row so compile() can resolve the opcode
        if name not in dv._SUB_OPCODE_FOR_NAME:
            dv._SUB_OPCODE_FOR_NAME[name] = max(dv._SUB_OPCODE_FOR_NAME.values()) + 1
        try:
            op.compile("v3")
        except ValueError as e:
            m = _re.search(r'uops_sha\["v3"\]="([0-9a-f]+)"', str(e))
            assert m, f"no sha in: {e}"
            op = dv.DveOp(name, spec, subdim=False, uops_sha={"v3": m.group(1)})
        op.compile("v3")
        dv.OPS.append(op)
        dv.CUSTOM_DVE_SPECS[name] = spec
        return op

    # T2: from s = q^2 (bounded [0, 0.345] because mish >= -0.309):
    # d = s+1; y0 = C0 + C1*d (linear minimax seed of 1/d on [1,1.345]);
    # y1 = y0*(C2 - d*y0) (one Newton step, C2=2); out = (1-s)*y1 = t2.
    def _refT(in0, in1, c0, c1, c2):
        d = in0 + np.float32(1.0)
        y0 = c0 + c1 * d
        y1 = y0 * (c2 - d * y0)
        return (np.float32(1.0) - in0) * y1

    _dT = Src0 + One
    _y0T = C0 + C1 * _dT
    _y1T = _y0T * (C2 - _dT * _y0T)
    bodyT = (One - Src0) * _y1T

    # P1/P2: one fused op per mish.  t = (1-s)/(1+s) with s = q^2 is
    # approximated by the minimax cubic Q(s) = 1 + s*(C0 + s*(C1 + s*C2))
    # (constant term pinned at 1), and the product with the pre-activation
    # is folded into the op's 8th ALU node: out = Src1 * Q(Src0^2).
    def _refP(in0, in1, c0, c1, c2):
        s = in0 * in0
        q = np.float32(1.0) + s * (c0 + s * (c1 + s * c2))
        return in1 * q

    _sP = Src0 * Src0
    _qP = One + _sP * (C0 + _sP * (C1 + _sP * C2))
    bodyP = Src1 * _qP

    _CUSTOM_OPS["A"] = _mk("MISH_RECIP_A", bodyA, _refA)
    _CUSTOM_OPS["B"] = _mk("MISH_RECIP_B", bodyB, _refB)
    _CUSTOM_OPS["M"] = _mk("MISH_MUL_BIAS", bodyM, _refM)
    _CUSTOM_OPS["T"] = _mk("MISH_T2_FAST", bodyT, _refT)
    _CUSTOM_OPS["P"] = _mk("MISH_POLY_MUL", bodyP, _refP)
    return _CUSTOM_OPS


SEED_C0 = -0.23549792
SEED_C1 = 2.0017324
T2_C0 = 1.7340084390144614
T2_C1 = -0.7434944237918216

# minimax cubics for t = (1-s)/(1+s), s = sigmoid(-x)^2 (see _refP):
# P1 on s in [0, 1] (first mish), P2 on s in [0, 0.334] (second mish,
# bounded because mish1 output >= -0.3125).
P1_C = (-1.924537766665479, 1.4214524363965337, -0.4999994122815388)
P2_C = (-1.9928074644613645, 1.8571218730908667, -1.1389181239973036)


def build_nc(
    identity_act=False,
    xw_bufs=2,
    pg_bufs=4,
    mid_bufs=3,
    out_bufs=2,
    skip_dma_out=False,
    yb_mode="biasmm",
    NB=2,
):
    """Build the single-core Bass program (SPMD across 8 cores).

    Per-group elementwise chain (group = 4 PSUM banks = 2032 outputs/part):
      q1 = sigmoid(-(z+b))   [ACT, reads PSUM]
      yb = z + b             [Pool tensor_scalar (or ACT / ACT+DVE split)]
      p  = yb * Q3(q1^2)     [DVE custom P1; Q3 ~ (1-s)/(1+s), so p = mish1]
      q2 = sigmoid(-p)       [ACT]
      out= p * Q3'(q2^2)     [DVE custom P2 -> strided outt = mish2]
    The q2/P2 tail of group g is emitted after the q1/yb head of group g+1
    so each engine's queue stays in dataflow order and PSUM frees early.
    """
    ops = _register_custom_ops()
    OPP = ops["P"]
    nc = bacc.Bacc("TRN2", target_bir_lowering=False, debug=False, num_devices=1)

    x_d = nc.dram_tensor("x", [CIN, H, W], F32R, kind="ExternalInput")
    wpe_d = nc.dram_tensor("wpe", [128, KHW, COUT], F32R, kind="ExternalInput")
    wpo_d = nc.dram_tensor("wpo", [128, KHW, COUT], F32R, kind="ExternalInput")
    wse_d = nc.dram_tensor("wse", [64, KHW, COUT], F32R, kind="ExternalInput")
    wso_d = nc.dram_tensor("wso", [64, KHW, COUT], F32R, kind="ExternalInput")
    bias_d = nc.dram_tensor("bias", [COUT, 1], F32, kind="ExternalInput")
    biasrow_d = nc.dram_tensor("biasrow", [1, COUT], F32R, kind="ExternalInput")
    onesrow_d = nc.dram_tensor("onesrow", [1, 512], F32R, kind="ExternalInput")
    y_d = nc.dram_tensor("y", [COUT, HO, WO], BF16, kind="ExternalOutput")

    y_ap = y_d.ap()

    with tile.TileContext(nc) as tc:
        with (
            tc.tile_pool(name="wpool", bufs=1) as wpool,
            tc.tile_pool(name="xpool", bufs=xw_bufs) as xpool,
            tc.tile_pool(name="ppool", bufs=pg_bufs, space="PSUM") as ppool,
            tc.tile_pool(name="mpool", bufs=mid_bufs) as mpool,
            tc.tile_pool(name="opool", bufs=out_bufs) as opool,
        ):
            # ---- constants ----
            wpe = wpool.tile([128, KHW, COUT], F32R, tag="wpe")
            wpo = wpool.tile([128, KHW, COUT], F32R, tag="wpo")
            wse = wpool.tile([64, KHW, COUT], F32R, tag="wse")
            wso = wpool.tile([128, KHW, COUT], F32R, tag="wso")  # data in parts 64:128
            bias = wpool.tile([COUT, 1], F32, tag="bias")
            biasrow = wpool.tile([1, COUT], F32R, tag="biasrow")
            ones = wpool.tile([1, 512], F32R, tag="ones")
            nbias = wpool.tile([COUT, 1], F32, tag="nbias")

            stages = []  # software-pipeline state, one entry per group

            chunk_tails = {}

            def load_chunk(ro0, split=None, tail_defer=False):
                # load chunk: input rows ro0 .. ro0+33, parity-split.
                # split=k loads row-pairs [0,k) first (so the first groups'
                # matmuls can start before the whole chunk arrives).
                xw = xpool.tile([128, NI, W], F32R, tag="xw")
                src = x_d.ap()[:, ro0 : ro0 + 2 * NI, :].rearrange(
                    "c (i two) w -> two c i w", two=2
                )
                pieces = [(0, NI)] if split is None else [(0, split), (split, NI)]
                if tail_defer:
                    chunk_tails[id(xw)] = (src, pieces[1:])
                    pieces = pieces[:1]
                for i0, i1 in pieces:
                    nc.sync.dma_start(xw[0:64, i0:i1, :], src[0][:, i0:i1, :])
                    nc.sync.dma_start(xw[64:128, i0:i1, :], src[1][:, i0:i1, :])
                return xw

            def load_chunk_tail(xw):
                src, pieces = chunk_tails.pop(id(xw))
                for i0, i1 in pieces:
                    nc.sync.dma_start(xw[0:64, i0:i1, :], src[0][:, i0:i1, :])
                    nc.sync.dma_start(xw[64:128, i0:i1, :], src[1][:, i0:i1, :])

            # wpe + the first slice of chunk 0 come first so the PE can
            # start within ~4us; everything else loads behind them.
            nc.sync.dma_start(wpe[:], wpe_d.ap())
            xw = load_chunk(CHUNKS[0], split=5)
            nc.sync.dma_start(wpo[:], wpo_d.ap())
            nc.sync.dma_start(wse[:], wse_d.ap())
            nc.sync.dma_start(wso[64:128, :, :], wso_d.ap())
            nc.sync.dma_start(bias[:], bias_d.ap())
            nc.sync.dma_start(biasrow[:], biasrow_d.ap())
            nc.sync.dma_start(ones[:], onesrow_d.ap())
            nc.vector.tensor_scalar_mul(nbias[:], bias[:], -1.0)
            R = 4 * NB  # output rows per (lo0, both-par) pair = outt rows
            warm = [True]
            for ci, ro0 in enumerate(CHUNKS):
                for lo0 in range(0, 32, R):
                    for par in (0, 1):  # even rows then odd rows
                        pg = ppool.tile([128, NB, 512], F32, tag="pg")
                        if warm[0]:
                            # ramp the PE p-state cheaply: the first ~36
                            # queued instructions are costed at the cold
                            # clock, so make them tiny (80-col) matmuls
                            # that still span >3us of engine time.
                            warm[0] = False
                            for _ in range(34):
                                nc.tensor.matmul(
                                    pg[:, 0, 0:32],
                                    wpe[:, 0, :],
                                    xw[:, 0:1, 0:32],
                                    start=True,
                                    stop=True,
                                )
                        for b in range(NB):  # NB blocks of 2 rows (stride 2)
                            if yb_mode == "biasmm" and b == NB - 1:
                                # bank NB-1: pre-load the bias into PSUM
                                # via a contract-1 matmul (bias row x ones)
                                # so this bank's chain reads (z+b) straight
                                # from PSUM and needs no yb copy pass.
                                nc.tensor.matmul(
                                    pg[:, b, 0:508],
                                    biasrow[0:1, :],
                                    ones[0:1, 0:508],
                                    start=True,
                                    stop=False,
                                )
                            lr = lo0 + par + 4 * b  # first output row of block
                            i = (lr - par) // 2  # row-pair index
                            # pair taps: even par: (kh0 lower, kh1 upper) @ i
                            #            odd  par: (kh1 lower, kh2 upper) @ i+1
                            ip = i if par == 0 else i + 1
                            # single tap: even par: kh2, lower @ i+1
                            #             odd  par: kh0, upper @ i
                            is_ = i + 1 if par == 0 else i
                            wp = wpe if par == 0 else wpo
                            bias_in_psum = yb_mode == "biasmm" and b == NB - 1
                            for kw in range(KHW):
                                nc.tensor.matmul(
                                    pg[:, b, 0:508],
                                    wp[:, kw, :],
                                    xw[:, ip : ip + 2, kw : kw + 254],
                                    start=(kw == 0 and not bias_in_psum),
                                    stop=False,
                                )
                            for kw in range(KHW):
                                if par == 0:
                                    lhsT = wse[:, kw, :]
                                    rhs = xw[0:64, is_ : is_ + 2, kw : kw + 254]
                                else:
                                    lhsT = wso[64:128, kw, :]
                                    rhs = xw[64:128, is_ : is_ + 2, kw : kw + 254]
                                nc.tensor.matmul(
                                    pg[:, b, 0:508],
                                    lhsT,
                                    rhs,
                                    start=False,
                                    stop=(kw == KHW - 1),
                                )
                        if par == 0:
                            outt = opool.tile([128, R * WO], BF16, tag="outt")
                        EW = NB * 508
                        pg_in = pg[:, :, 0:508]
                        dst = outt[:].rearrange(
                            "p (s2 t w) -> p s2 t w", t=2, w=WO
                        )[:, :, par, :]
                        if identity_act:
                            # debug path: conv+bias only
                            nc.scalar.activation(
                                dst, pg_in, AFT.Identity, bias=bias[:]
                            )
                            continue
                        # ---- head of this group's elementwise chain ----
                        # yb = z+b for bank 0 via ACT identity (the only
                        # PSUM reader for that bank, so it frees early).
                        # In biasmm mode the last bank already holds z+b in
                        # PSUM (bias pre-loaded by the PE), so q1/P1 read it
                        # directly and no copy pass is needed.
                        ban = NB - 1 if yb_mode == "biasmm" else NB
                        yb = mpool.tile([128, ban * 508], F32, tag="yb")
                        q1 = mpool.tile([128, EW], F32, tag="q")
                        if yb_mode == "split":
                            nc.scalar.activation(
                                yb[:, 0 : (NB - 1) * 508],
                                pg[:, 0 : NB - 1, 0:508],
                                AFT.Identity,
                                bias=bias[:],
                            )
                            nc.vector.tensor_scalar_add(
                                yb[:, (NB - 1) * 508 :],
                                pg[:, NB - 1 : NB, 0:508],
                                bias[:],
                            )
                            nc.scalar.activation(
                                q1[:], yb[:], AFT.Sigmoid, scale=-1.0
                            )
                        elif yb_mode == "act":
                            nc.scalar.activation(
                                yb[:], pg_in, AFT.Identity, bias=bias[:]
                            )
                            nc.scalar.activation(
                                q1[:], yb[:], AFT.Sigmoid, scale=-1.0
                            )
                        else:  # "biasmm"
                            nc.scalar.activation(
                                yb[:], pg[:, 0:ban, 0:508],
                                AFT.Identity, bias=bias[:],
                            )
                            nc.scalar.activation(
                                q1[:, 0 : ban * 508], yb[:],
                                AFT.Sigmoid, scale=-1.0,
                            )
                            nc.scalar.activation(
                                q1[:, ban * 508 : EW],
                                pg[:, ban:NB, 0:508],
                                AFT.Sigmoid, scale=-1.0,
                            )

                        # ---- software pipeline: emit deferred stages so
                        # every cross-engine dependency is >= 1 cycle old
                        # (P1 one cycle late, q2 two, P2+DMA three) ----
                        def st_p1(q1=q1, yb=yb, pg=pg, ban=ban):
                            p = mpool.tile([128, EW], F32, tag="p")
                            nc.vector._custom_dve(
                                OPP,
                                out=p[:, 0 : ban * 508],
                                in0=q1[:, 0 : ban * 508],
                                in1=yb[:],
                                s0=P1_C[0], s1=P1_C[1], imm2=P1_C[2],
                            )
                            if ban < NB:
                                nc.vector._custom_dve(
                                    OPP,
                                    out=p[:, ban * 508 : EW],
                                    in0=q1[:, ban * 508 : EW],
                                    in1=pg[:, ban, 0:508],
                                    s0=P1_C[0], s1=P1_C[1], imm2=P1_C[2],
                                )
                            return p

                        def st_q2(p):
                            q2 = mpool.tile([128, EW], F32, tag="q2")
                            nc.scalar.activation(
                                q2[:], p[:], AFT.Sigmoid, scale=-1.0
                            )
                            return q2

                        def st_p2(p, q2, dst=dst, outt=outt,
                                  par=par, ro0=ro0, lo0=lo0):
                            nc.vector._custom_dve(
                                OPP, out=dst, in0=q2[:], in1=p[:],
                                s0=P2_C[0], s1=P2_C[1], imm2=P2_C[2],
                            )
                            if par == 1 and not skip_dma_out:
                                r0 = ro0 + lo0
                                nc.sync.dma_start(
                                    y_ap[:, r0 : r0 + R, :], outt[:]
                                )

                        stages.append({"p1": st_p1, "q2": st_q2, "p2": st_p2})

                        def advance():
                            # run stage k of the group k cycles back
                            n = len(stages)
                            if n >= 2 and "p" not in stages[-2]:
                                g = stages[-2]
                                g["p"] = g["p1"]()
                            if n >= 3 and "q2v" not in stages[-3]:
                                g = stages[-3]
                                g["q2v"] = g["q2"](g["p"])
                            if n >= 4 and "done" not in stages[-4]:
                                g = stages[-4]
                                g["p2"](g["p"], g["q2v"])
                                g["done"] = True

                        advance()
                        if lo0 == 0 and par == 0 and ci + 1 < len(CHUNKS):
                            # prefetch next chunk's input while this chunk
                            # computes (ahead of this chunk's out-DMAs in
                            # the queue, so PE never waits at the boundary)
                            xw_next = load_chunk(CHUNKS[ci + 1])
                xw = xw_next
            # drain the pipeline (3 more virtual cycles)
            for _ in range(3):
                stages.append({})
                n = len(stages)
                if "p1" in stages[-2] and "p" not in stages[-2]:
                    g = stages[-2]
                    g["p"] = g["p1"]()
                if n >= 3 and "q2" in stages[-3] and "q2v" not in stages[-3]:
                    g = stages[-3]
                    g["q2v"] = g["q2"](g["p"])
                if n >= 4 and "p2" in stages[-4] and "done" not in stages[-4]:
                    g = stages[-4]
                    g["p2"](g["p"], g["q2v"])
                    g["done"] = True

    nc.compile()
    return nc


def pack_inputs(x, weight, bias_v):
    """Host-side packing: fp32r rounding + weight tap stacking + per-core x."""
    x = np.ascontiguousarray(np.asarray(x, dtype=np.float32))
    weight = np.ascontiguousarray(np.asarray(weight, dtype=np.float32))
    bias_v = np.ascontiguousarray(np.asarray(bias_v, dtype=np.float32))

    wr = fp32r_round(weight)  # [cout, cin, kh, kw]
    wT = wr.transpose(1, 0, 2, 3)  # [cin, cout, kh, kw]

    def lhsT(kh):  # [cin, kw, cout] -> slice per kw gives [cin, cout]
        return np.ascontiguousarray(wT[:, :, kh, :].transpose(0, 2, 1))

    k0, k1, k2 = lhsT(0), lhsT(1), lhsT(2)
    wpe = np.concatenate([k0, k1], axis=0)  # even pairs: kh0 lower, kh1 upper
    wpo = np.concatenate([k1, k2], axis=0)  # odd pairs:  kh1 lower, kh2 upper
    wse = k2  # even single: kh2, lower
    wso = k0  # odd single:  kh0, upper

    xr = fp32r_round(x)
    common = {
        "wpe": wpe,
        "wpo": wpo,
        "wse": wse,
        "wso": wso,
        "bias": bias_v.reshape(COUT, 1),
        "biasrow": fp32r_round(bias_v.reshape(1, COUT)),
        "onesrow": np.ones((1, 512), dtype=np.float32),
    }
    in_maps = [
        dict(common, x=np.ascontiguousarray(xr[n])) for n in range(xr.shape[0])
    ]
    return in_maps


_NC_CACHE = {}


def _get_nc():
    if "nc" not in _NC_CACHE:
        _NC_CACHE["nc"] = build_nc()
    return _NC_CACHE["nc"]


def kernel(x, weight, bias):
    nc = _get_nc()
    in_maps = pack_inputs(x, weight, bias)
    res = run_bass_kernel_spmd(nc, in_maps, core_ids=list(range(NCORES)))
    y = np.stack(
        [
            np.asarray(res.results[n]["y"]).astype(np.float32)
            for n in range(NCORES)
        ],
        axis=0,
    )
    return y

